# revision 2
# baseline (speedup 1.0000x reference)
# MoE top-2 routing kernel for Trainium2, 8 NeuronCores, data-parallel over batch.
# v7: software-pipelined repeat iterations — instruction emission interleaves
# iteration i's expert/combine phase with iteration i+1's routing/dispatch
# phase (in-order engine queues require emission-order interleaving), with
# ping-pong DRAM scratch by iteration parity.
# Self-contained: hardcodes shapes B=8, S=2048, D=1024, E=8, TOP_K=2.
import numpy as np

B, S, D, E = 8, 2048, 1024, 8
TOPK = 2
P = 128
CAP_TILES = 5            # per-expert slot capacity in 128-token tiles
CAP = CAP_TILES * P      # 640
NSLOT = E * CAP          # 5120
NKT = D // P             # 8 contraction tiles


def build_kernel(s_local=S, cap_tiles=CAP_TILES, repeat=1, interleave=True):
    """Build the per-core Bass module. s_local = tokens per core."""
    import dataclasses as _dc
    import contextlib
    import concourse.bacc as bacc
    import concourse.tile as tile
    import concourse.mybir as mybir
    import concourse.bass as bass
    from concourse.masks import make_identity

    dt = mybir.dt
    cap = cap_tiles * P
    nslot = E * cap
    nt = s_local // P          # token tiles
    ncol = 2 * nt              # dispatch-entry columns (k-major, then tile)

    nc = bacc.Bacc(None, target_bir_lowering=False, debug=False,
                   dynamic_dma_scratch_size=16384)

    Xd = nc.declare_dram_parameter("X", [s_local, D], dt.float32, isOutput=False)
    Ad = nc.declare_dram_parameter("A", [E, P, NKT, D], dt.bfloat16, isOutput=False)
    bP9d = nc.declare_dram_parameter("bP9", [E + 1, D], dt.float32, isOutput=False)
    WrTd = nc.declare_dram_parameter("WrT", [P, NKT, E], dt.float32, isOutput=False)
    brd = nc.declare_dram_parameter("br", [1, E], dt.float32, isOutput=False)
    outd = nc.declare_dram_parameter("out", [s_local, D], dt.float32, isOutput=True)

    def bcast8(apobj):
        return _dc.replace(apobj, ap=[[0, 8]] + list(apobj.ap))

    XsDs = [nc.dram_tensor(f"xs_scratch{i}", [nslot, D], dt.bfloat16)
            for i in range(2)]
    ZbDs = [nc.dram_tensor(f"z_scratch{i}", [nslot, D], dt.bfloat16)
            for i in range(2)]
    skDs = [nc.dram_tensor(f"sk_scratch{i}", [TOPK, 16, s_local // 16], dt.int16)
            for i in range(2)]
    idx0D = nc.dram_tensor("idx0_scratch", [16, cap // 16], dt.int16)

    fp32 = dt.float32
    bf16 = dt.bfloat16

    with tile.TileContext(nc) as tc:
        with contextlib.ExitStack() as ctx:
            const_p = ctx.enter_context(tc.tile_pool(name="const", bufs=1))
            rt_p = ctx.enter_context(tc.tile_pool(name="routing", bufs=1))
            ps_p = ctx.enter_context(tc.tile_pool(name="psum", bufs=2, space="PSUM"))
            mm_p = ctx.enter_context(tc.tile_pool(name="psmm", bufs=2, space="PSUM"))
            pst_p = ctx.enter_context(tc.tile_pool(name="psumt", bufs=2, space="PSUM"))

            # ---------- constants ----------
            ID = const_p.tile([P, P], fp32)
            make_identity(nc, ID[:])
            # U[p, m] = 1.0 iff p < m (strict upper): exclusive prefix
            U = const_p.tile([P, P], fp32)
            nc.gpsimd.memset(U[:], 1.0)
            nc.gpsimd.affine_select(
                out=U[:], in_=U[:], compare_op=mybir.AluOpType.is_gt,
                fill=0.0, base=0, channel_multiplier=-1, pattern=[[1, P]],
            )
            ones_col = const_p.tile([P, 1], fp32)
            nc.gpsimd.memset(ones_col[:], 1.0)
            ones_row = const_p.tile([1, P], fp32)
            nc.gpsimd.memset(ones_row[:], 1.0)
            IDb = const_p.tile([P, P], bf16)
            nc.vector.tensor_copy(out=IDb[:], in_=ID[:])

            WrTs = const_p.tile([P, NKT, E], fp32)
            nc.sync.dma_start(out=WrTs[:], in_=WrTd[:])
            brS = const_p.tile([1, E], fp32)
            nc.sync.dma_start(out=brS[:], in_=brd[:])
            bP9b = const_p.tile([E + 1, D], bf16)

            # gather identity indices (wrapped-16 layout) for per-expert gathers
            i16 = const_p.tile([16, cap // 16], dt.int16)
            nc.gpsimd.iota(i16[:], pattern=[[16, cap // 16]], base=0,
                           channel_multiplier=1)
            nc.sync.dma_start(out=idx0D[:], in_=i16[:])
            IDX0 = const_p.tile([P, cap // 16], dt.int16)
            nc.sync.dma_start(out=IDX0[:], in_=bcast8(idx0D[:]))

            ZROW = 2                      # rows per partition in one zero DMA
            zt = const_p.tile([P, ZROW * D], bf16)
            nc.gpsimd.memset(zt[:], 0.0)

            # persistent pools; bufs=2 tags rotate across iterations
            xf_p = ctx.enter_context(tc.tile_pool(name="xf", bufs=2))
            bP9f = xf_p.tile([P, D], fp32, tag="xf", name="bP9f")
            nc.sync.dma_start(out=bP9f[:E + 1, :], in_=bP9d[:])
            nc.vector.tensor_copy(out=bP9b[:], in_=bP9f[:E + 1, :])
            xt_p = ctx.enter_context(tc.tile_pool(name="xt", bufs=2))
            sm_p = ctx.enter_context(tc.tile_pool(name="sm", bufs=4))
            xw_p = ctx.enter_context(tc.tile_pool(name="xw", bufs=1))
            we_p = ctx.enter_context(tc.tile_pool(name="we", bufs=2))
            gt_p = ctx.enter_context(tc.tile_pool(name="gt", bufs=2))
            zc_p = ctx.enter_context(tc.tile_pool(name="zc", bufs=3))
            tk_p = ctx.enter_context(tc.tile_pool(name="tkp", bufs=2))
            oc_p = ctx.enter_context(tc.tile_pool(name="ocp", bufs=3))

            def front(rep):
                """Routing + dispatch build + scatter. Yields between chunks;
                final value via StopIteration is the state dict for back()."""
                XsD = XsDs[rep % 2]
                skD = skDs[rep % 2]
                XW = xw_p.tile([P, ncol, D], bf16, tag="xw", name="XW")
                ENT = rt_p.tile([P, ncol], fp32, tag="ent", bufs=2, name="ENT")
                WSel = rt_p.tile([P, ncol], fp32, tag="wsel", bufs=2, name="WSel")
                W9T = rt_p.tile([E + 1, nt, P], bf16, tag="w9t", bufs=2,
                                name="W9T")

                # ---------- phase 1: load X, router, top-2, build XW ----------
                for t in range(nt):
                    if t == 4:
                        # zero-fill XsD (scatter-add accumulates); deferred
                        # so the first X loads stream unimpeded
                        zspan = ZROW * P
                        for z0 in range(0, nslot, zspan):
                            nc.sync.dma_start(
                                out=XsD[z0:z0 + zspan, :].rearrange(
                                    "(p c) d -> p (c d)", c=ZROW),
                                in_=zt[:])
                    xf = xf_p.tile([P, D], fp32, tag="xf", name="xf")
                    nc.sync.dma_start(out=xf[:], in_=Xd[t * P:(t + 1) * P, :])
                    XTt = xt_p.tile([P, NKT, P], fp32, name="XTt")
                    for g4 in range(2):
                        pt4 = pst_p.tile([P, 4 * P], fp32, space="PSUM",
                                         tag="tr", name="pt4")
                        for q in range(4):
                            kt = g4 * 4 + q
                            nc.tensor.transpose(
                                out=pt4[:, q * P:(q + 1) * P],
                                in_=xf[:, kt * P:(kt + 1) * P],
                                identity=ID[:])
                        dst = XTt[:, g4 * 4:(g4 + 1) * 4, :].rearrange(
                            "p a b -> p (a b)")
                        if g4 == 0:
                            nc.vector.tensor_copy(out=dst, in_=pt4[:])
                        else:
                            nc.scalar.activation(
                                out=dst, in_=pt4[:],
                                func=mybir.ActivationFunctionType.Copy)
                    lg = ps_p.tile([P, E], fp32, space="PSUM", tag="dsp",
                                   name="lg")
                    for kt in range(NKT):
                        nc.tensor.matmul(
                            out=lg[:], lhsT=XTt[:, kt, :], rhs=WrTs[:, kt, :],
                            start=(kt == 0), stop=False)
                    nc.tensor.matmul(
                        out=lg[:], lhsT=ones_row[:], rhs=brS[:],
                        start=False, stop=True)

                    # softmax without max-subtraction (logits are O(1))
                    Eexp = sm_p.tile([P, E], fp32, tag="eexp", name="Eexp")
                    Zs = sm_p.tile([P, 1], fp32, tag="zs", name="Zs")
                    nc.scalar.activation(
                        out=Eexp[:], in_=lg[:],
                        func=mybir.ActivationFunctionType.Exp,
                        scale=1.0, accum_out=Zs[:, 0:1])
                    rZ = sm_p.tile([P, 1], fp32, tag="rz", name="rZ")
                    nc.vector.reciprocal(out=rZ[:], in_=Zs[:])
                    Wsm = sm_p.tile([P, E], fp32, tag="wsm", name="Wsm")
                    nc.vector.tensor_scalar_mul(Wsm[:], Eexp[:], rZ[:, 0:1])
                    Wm8 = sm_p.tile([P, E], fp32, tag="wm8", name="Wm8")
                    nc.vector.max(out=Wm8[:], in_=Wsm[:])
                    Wi8 = sm_p.tile([P, E], dt.uint32, tag="wi8", name="Wi8")
                    nc.vector.max_index(out=Wi8[:], in_max=Wm8[:], in_values=Wsm[:])
                    # off-chain bookkeeping on gpsimd
                    IdxF = sm_p.tile([P, E], fp32, tag="idxf", name="IdxF")
                    nc.gpsimd.tensor_copy(out=IdxF[:, :TOPK], in_=Wi8[:, :TOPK])
                    nc.gpsimd.tensor_copy(out=ENT[:, t:t + 1], in_=IdxF[:, 0:1])
                    nc.gpsimd.tensor_copy(
                        out=ENT[:, nt + t:nt + t + 1], in_=IdxF[:, 1:2])
                    nc.gpsimd.tensor_copy(out=WSel[:, t:t + 1], in_=Wm8[:, 0:1])
                    nc.gpsimd.tensor_copy(
                        out=WSel[:, nt + t:nt + t + 1], in_=Wm8[:, 1:2])
                    # scaled entry rows for the dispatch scatter
                    nc.gpsimd.tensor_scalar(
                        out=XW[:, t, :], in0=xf[:],
                        scalar1=Wm8[:, 0:1], scalar2=None,
                        op0=mybir.AluOpType.mult)
                    nc.scalar.activation(
                        out=XW[:, nt + t, :], in_=xf[:],
                        func=mybir.ActivationFunctionType.Copy,
                        scale=Wm8[:, 1:2])
                    # masked top-2 weights + ones column -> transposed [9, 128]
                    msk = sm_p.tile([P, E], fp32, tag="msk", name="msk")
                    nc.vector.tensor_scalar(
                        out=msk[:], in0=Wsm[:], scalar1=Wm8[:, 1:2],
                        scalar2=None, op0=mybir.AluOpType.is_ge)
                    w9 = sm_p.tile([P, E + 1], fp32, tag="w9", name="w9")
                    nc.vector.tensor_mul(out=w9[:, :E], in0=Wsm[:], in1=msk[:])
                    nc.vector.memset(w9[:, E:], 1.0)
                    w9tp = pst_p.tile([E + 1, P], fp32, space="PSUM", tag="tr",
                                      name="w9tp")
                    nc.tensor.transpose(out=w9tp[:], in_=w9[:], identity=ID[:])
                    nc.vector.tensor_copy(out=W9T[:, t, :], in_=w9tp[:])
                    yield

                # ---------- phase 2: dispatch build ----------
                Ms = []
                for e in range(E):
                    Me = sm_p.tile([P, ncol], fp32, tag=f"m{e}", bufs=2,
                                   name="Me")
                    nc.vector.tensor_scalar(
                        out=Me[:], in0=ENT[:], scalar1=float(e), scalar2=None,
                        op0=mybir.AluOpType.is_equal)
                    Ms.append(Me)

                Sp = ps_p.tile([ncol, E], fp32, space="PSUM", tag="dsp",
                               name="Sp")
                for e in range(E):
                    nc.tensor.matmul(out=Sp[:, e:e + 1], lhsT=Ms[e][:],
                                     rhs=ones_col[:], start=True, stop=True)
                Ssb = sm_p.tile([ncol, E], fp32, tag="ssb", name="Ssb")
                nc.vector.tensor_copy(out=Ssb[:], in_=Sp[:])
                CSp = ps_p.tile([ncol, E], fp32, space="PSUM", tag="dsp",
                                name="CSp")
                nc.tensor.matmul(out=CSp[:], lhsT=U[:ncol, :ncol], rhs=Ssb[:],
                                 start=True, stop=True)
                CSsb = sm_p.tile([ncol, E], fp32, tag="cssb", name="CSsb")
                nc.vector.tensor_copy(out=CSsb[:], in_=CSp[:])
                yield
                CSTrows = []
                for e in range(E):
                    cstp = ps_p.tile([1, ncol], fp32, space="PSUM", tag="dsp",
                                     name="cstp")
                    nc.tensor.transpose(
                        out=cstp[:], in_=CSsb[:, e:e + 1],
                        identity=ID[:ncol, :ncol])
                    cstr = sm_p.tile([1, ncol], fp32, tag=f"cst{e}", bufs=2,
                                     name="cstr")
                    nc.vector.tensor_copy(out=cstr[:], in_=cstp[:])
                    CSTrows.append(cstr)

                SLOT = rt_p.tile([P, ncol], fp32, tag="slot", bufs=2,
                                 name="SLOT")
                nc.vector.tensor_scalar(
                    out=SLOT[:], in0=ENT[:], scalar1=float(cap), scalar2=None,
                    op0=mybir.AluOpType.mult)
                yield
                for e in range(E):
                    Rp = ps_p.tile([P, ncol], fp32, space="PSUM", tag="dsp",
                                   name="Rp")
                    nc.tensor.matmul(out=Rp[:], lhsT=U[:], rhs=Ms[e][:],
                                     start=True, stop=False)
                    nc.tensor.matmul(out=Rp[:], lhsT=ones_row[:],
                                     rhs=CSTrows[e][:], start=False, stop=True)
                    tmp = sm_p.tile([P, ncol], fp32, tag="rtmp", name="tmp")
                    nc.vector.tensor_mul(out=tmp[:], in0=Ms[e][:], in1=Rp[:])
                    nc.vector.tensor_add(out=SLOT[:], in0=SLOT[:], in1=tmp[:])
                    if e % 3 == 2:
                        yield

                # slot-of-entry index tiles (wrapped-16)
                slottp = ps_p.tile([ncol, P], fp32, space="PSUM", tag="dsp",
                                   name="slottp")
                nc.tensor.transpose(out=slottp[:], in_=SLOT[:], identity=ID[:])
                SLOTT16 = sm_p.tile([ncol, P], dt.int16, tag="slott16",
                                    name="SLOTT16")
                nc.vector.tensor_copy(out=SLOTT16[:], in_=slottp[:])
                for k in range(TOPK):
                    nc.sync.dma_start(
                        out=skD[k].rearrange("p (t h) -> t h p", h=8),
                        in_=SLOTT16[k * nt:(k + 1) * nt, :].rearrange(
                            "t (h p) -> t h p", p=16))
                SK = rt_p.tile([P, ncol * P // 16], dt.int16, tag="sk", bufs=2,
                               name="SK")
                hw_half = s_local // 16
                for k in range(TOPK):
                    nc.sync.dma_start(
                        out=SK[:, k * hw_half:(k + 1) * hw_half],
                        in_=bcast8(skD[k][:]))
                yield

                # ---------- phase 3: one scatter-add dispatches all entries ----
                nc.gpsimd.dma_scatter_add(
                    out_ap=XsD[:], in_ap=XW[:], idxs_ap=SK[:],
                    num_idxs=ncol * P, num_idxs_reg=ncol * P, elem_size=D)
                return_state = {"SK": SK, "W9T": W9T, "rep": rep}
                yield return_state

            def back(st):
                """Expert matmuls + combine for a completed front state."""
                rep = st["rep"]
                XsD = XsDs[rep % 2]
                ZbD = ZbDs[rep % 2]
                SK, W9T = st["SK"], st["W9T"]
                hw_half = s_local // 16

                # ---------- phase 4: per-expert gather + expert matmul ----------
                for e in range(E):
                    Ae = we_p.tile([P, NKT, D], bf16, tag="ae", name="Ae")
                    nc.sync.dma_start(out=Ae[:], in_=Ad[e])
                    idxe = gt_p.tile([P, cap // 16], dt.int16, tag="idxe",
                                     name="idxe")
                    nc.vector.tensor_scalar(
                        out=idxe[:], in0=IDX0[:], scalar1=e * cap, scalar2=None,
                        op0=mybir.AluOpType.add)
                    gt = gt_p.tile([P, NKT, cap], bf16, tag="gt", name="gt")
                    nc.gpsimd.dma_gather(
                        out_ap=gt[:], in_ap=XsD[:], idxs_ap=idxe[:],
                        num_idxs=cap, num_idxs_reg=cap, elem_size=D,
                        transpose=True)
                    for r in range(cap_tiles):
                        zp = mm_p.tile([P, D], fp32, space="PSUM", tag="mm",
                                       name="zp")
                        for kt in range(NKT):
                            for h2 in range(2):
                                hsl = slice(h2 * 512, (h2 + 1) * 512)
                                nc.tensor.matmul(
                                    out=zp[:, hsl],
                                    lhsT=gt[:, kt, r * P:(r + 1) * P],
                                    rhs=Ae[:, kt, hsl],
                                    start=(kt == 0), stop=(kt == NKT - 1))
                        zsb = zc_p.tile([P, D], bf16, tag="zsb", name="zsb")
                        if r % 2 == 0:
                            nc.vector.tensor_copy(out=zsb[:], in_=zp[:])
                        else:
                            nc.scalar.activation(
                                out=zsb[:], in_=zp[:],
                                func=mybir.ActivationFunctionType.Copy)
                        nc.sync.dma_start(
                            out=ZbD[e * cap + r * P: e * cap + (r + 1) * P, :],
                            in_=zsb[:])
                    yield

                # ---------- phase 5: combine + bias ----------
                GCH = 512
                nch = s_local // GCH
                for c in range(nch):
                    Tkc = []
                    for k in range(TOPK):
                        tkb = tk_p.tile([P, GCH // P, D], bf16, tag=f"tk{k}",
                                        name="tkb")
                        nc.gpsimd.dma_gather(
                            out_ap=tkb[:], in_ap=ZbD[:],
                            idxs_ap=SK[:, k * hw_half + c * (GCH // 16):
                                       k * hw_half + (c + 1) * (GCH // 16)],
                            num_idxs=GCH, num_idxs_reg=GCH, elem_size=D,
                            transpose=False)
                        Tkc.append(tkb)
                    for c2 in range(GCH // P):
                        T = c * (GCH // P) + c2
                        pb = mm_p.tile([P, D], fp32, space="PSUM", tag="mm",
                                       name="pb")
                        for h2 in range(2):
                            hsl = slice(h2 * 512, (h2 + 1) * 512)
                            # bias + Tk0 + Tk1 accumulated on the PE via
                            # identity matmuls; out copied from PSUM
                            nc.tensor.matmul(
                                out=pb[:, hsl], lhsT=W9T[:, T, :],
                                rhs=bP9b[:, hsl], start=True, stop=False)
                            nc.tensor.matmul(
                                out=pb[:, hsl], lhsT=IDb[:],
                                rhs=Tkc[0][:, c2, hsl], start=False, stop=False)
                            nc.tensor.matmul(
                                out=pb[:, hsl], lhsT=IDb[:],
                                rhs=Tkc[1][:, c2, hsl], start=False, stop=True)
                        osb = oc_p.tile([P, D], fp32, tag="osb", name="osb")
                        if c2 % 2 == 0:
                            nc.vector.tensor_copy(out=osb[:], in_=pb[:])
                        else:
                            nc.scalar.activation(
                                out=osb[:], in_=pb[:],
                                func=mybir.ActivationFunctionType.Copy)
                        nc.sync.dma_start(
                            out=outd[T * P:(T + 1) * P, :], in_=osb[:])
                    yield

            def drain(gen):
                state = None
                for v in gen:
                    if v is not None:
                        state = v
                return state

            if not interleave or repeat == 1:
                for rep in range(repeat):
                    st = drain(front(rep))
                    drain(back(st))
            else:
                st = drain(front(0))
                for rep in range(repeat):
                    b = back(st)
                    f = front(rep + 1) if rep + 1 < repeat else None
                    st = None
                    alive = True
                    while alive:
                        alive = False
                        try:
                            next(b)
                            alive = True
                        except StopIteration:
                            pass
                        if f is not None:
                            for _ in range(2):
                                try:
                                    v = next(f)
                                    if v is not None:
                                        st = v
                                    alive = True
                                except StopIteration:
                                    break

    nc.compile()
    return nc


_NC_CACHE = {}


def _get_nc(s_local=S, cap_tiles=CAP_TILES):
    key = (s_local, cap_tiles)
    if key not in _NC_CACHE:
        _NC_CACHE[key] = build_kernel(s_local, cap_tiles)
    return _NC_CACHE[key]


def make_in_maps(X, We, be, Wr, br, Wo, bo):
    import ml_dtypes
    bf = ml_dtypes.bfloat16
    We = np.asarray(We, np.float32)
    Wo = np.asarray(Wo, np.float32)
    # A[e] = We[e]^T @ Wo^T  [d, h_out]; pre-tiled to [E, 128, NKT, D]
    A = np.matmul(We.transpose(0, 2, 1), Wo.T)                  # [E, D, D]
    A = np.ascontiguousarray(
        A.reshape(E, NKT, P, D).transpose(0, 2, 1, 3)).astype(bf)
    bP9 = np.concatenate(
        [np.asarray(be, np.float32) @ Wo.T,
         np.asarray(bo, np.float32).reshape(1, D)], axis=0)     # [9, D]
    Wr = np.asarray(Wr, np.float32)
    WrT = np.ascontiguousarray(Wr.T.reshape(NKT, P, E).transpose(1, 0, 2))
    brC = np.ascontiguousarray(np.asarray(br, np.float32).reshape(1, E))
    Xc = np.asarray(X, np.float32)
    return [
        {"X": np.ascontiguousarray(Xc[c]), "A": A, "bP9": bP9,
         "WrT": WrT, "br": brC}
        for c in range(B)
    ]


def kernel(X, We, be, Wr, br, Wo, bo):
    from concourse.bass_utils import run_bass_kernel_spmd
    nc = _get_nc()
    in_maps = make_in_maps(X, We, be, Wr, br, Wo, bo)
    res = run_bass_kernel_spmd(nc, in_maps, list(range(B)))
    out = np.stack([res.results[c]["out"] for c in range(B)], axis=0)
    return out.astype(np.float32)
